# revision 19
# baseline (speedup 1.0000x reference)
"""Trainium2 Bass kernel for nn_BackgroundEncoder.

Data-parallel over bz across 8 cores (2 samples/core). Inside each core:
  FiLM -> iFFT(cos matmul) -> 15-step KV-cached decoder -> masked encoder layer.
All math fp32. Host pre-transposes/folds weights and gathers output.
"""
import math
import sys

import numpy as np

sys.path.insert(0, "/opt/trn_rl_repo")

import concourse.bass as bass
import concourse.bacc as bacc
import concourse.mybir as mybir
from concourse.bass_utils import run_bass_kernel_spmd
from concourse.tile import TileContext

F32 = mybir.dt.float32
AX = mybir.AxisListType
OP = mybir.AluOpType
AF = mybir.ActivationFunctionType

POS = 256
FEAT = 7
BZ = 16
SL = 128
T = 16
H = 8
DH = 32
FF = 1024
EH = 4
EDH = 64
NCORES = 8
BL = 2
S = BL * SL
NSEQ = BL * T
SP = SL + 1
NCOL = NSEQ * SP
NEG = -30000.0
EPS = 1e-5

_CACHE = {}


_PSPEC = [
    ("xe_tok", (S, POS)), ("fT", (FEAT, S)), ("se_wT", (FEAT, 2 * POS)),
    ("se_bg", (128, POS)), ("se_bb", (128, POS)), ("cosC", (SL, SL)),
    ("ident", (128, 128)), ("ones128", (128, 1)), ("onesr", (1, 128)),
    ("eps_ap", (128, 1)), ("zcol", (128, 1)), ("maskA", (SL, NSEQ)),
    ("WqT", (POS, POS)), ("WkT", (POS, POS)), ("WvT", (POS, POS)),
    ("WoT", (POS, POS)), ("bq_rep", (128, POS)), ("bk_rep", (128, POS)),
    ("bv_rep", (128, POS)), ("bo_rep", (128, POS)), ("WvcaT", (POS, POS)),
    ("bvca", (POS, 1)), ("WocaT", (POS, POS)), ("cab_rep", (128, POS)),
    ("g1_rep", (128, POS)), ("W1T", (POS, FF)), ("b1ff", (FF, 1)),
    ("g2_rep", (128, POS)), ("W2T", (FF, POS)), ("b2_rep", (128, POS)),
    ("g3_rep", (128, POS)), ("b3_rep", (128, POS)), ("gon_rep", (128, POS)),
    ("bon_rep", (128, POS)), ("WeqT", (POS, POS)), ("WekT", (POS, POS)),
    ("WevT", (POS, POS)), ("WeoT", (POS, POS)), ("beq", (POS, 1)),
    ("bek", (POS, 1)), ("bev_rep", (128, POS)), ("beo", (POS, 1)),
    ("We1T", (POS, POS)), ("be1ff", (POS, 1)), ("We2T", (POS, POS)),
    ("be2ff", (POS, 1)),
]


def _build_pack(vals):
    lay, total = _playout()
    a = np.zeros((128, total), np.float32)
    for name, (r, c) in _PSPEC:
        col, rr, cc, n = lay[name]
        v = np.asarray(vals[name], np.float32)
        if n == 1:
            a[0:r, col:col + c] = v
        else:
            for i in range(n):
                a[:, col + i * c:col + (i + 1) * c] = v[i * 128:(i + 1) * 128]
    return a


def _playout():
    lay, cur = {}, 0
    for name, (r, c) in _PSPEC:
        n = 1 if r <= 128 else r // 128
        lay[name] = (cur, r, c, n)
        cur += c * n
    return lay, cur


class _Pack:
    def __init__(self):
        self.lay, self.total = _playout()
        self.tile = None

    def get(self, name):
        col, r, c, n = self.lay[name]
        if n == 1:
            return self.tile[0:min(r, 128), col:col + c]
        return [self.tile[:, col + i * c:col + (i + 1) * c] for i in range(n)]


def _c(tc, pool, name, shape):
    return tc._pack.get(name)


def build_kernel():
    nc = bacc.Bacc("TRN2", target_bir_lowering=False)
    out_d = nc.dram_tensor("out_T", [POS, NCOL], F32, kind="ExternalOutput")
    goal_d = nc.dram_tensor("goalT_bc", [POS, BL, T], F32, kind="ExternalInput")

    with TileContext(nc) as tc:
        V = nc.vector
        G = nc.gpsimd
        A = nc.scalar
        PE = nc.tensor

        ctx_con = tc.tile_pool(name="con", bufs=1)
        con = ctx_con.__enter__()
        pack = _Pack()
        tc._pack = pack
        cpack_d = nc.dram_tensor("cpack", [128, pack.total], F32,
                                 kind="ExternalInput")
        cpack = con.tile([128, pack.total], F32, name="cpack")
        nc.gpsimd.dma_start(cpack[:, :], cpack_d[:, :])
        pack.tile = cpack
        warm = con.tile([128, 3], F32, name="warm")
        nc.vector.tensor_copy(warm[:, 0:1], cpack[:, 0:1])
        nc.scalar.activation(warm[:, 1:2], cpack[:, 0:1], AF.Identity,
                             bias=cpack[:, 0:1])
        nc.gpsimd.tensor_copy(warm[:, 2:3], cpack[:, 0:1])
        xe = _c(tc, con, "xe_tok", (S, POS))
        fT = _c(tc, con, "fT", (FEAT, S))
        se_wT = _c(tc, con, "se_wT", (FEAT, 2 * POS))
        se_bg = _c(tc, con, "se_bg", (128, POS))
        se_bb = _c(tc, con, "se_bb", (128, POS))
        cosC = _c(tc, con, "cosC", (SL, SL))
        ident = _c(tc, con, "ident", (128, 128))
        ones128 = _c(tc, con, "ones128", (128, 1))
        onesr = _c(tc, con, "onesr", (1, 128))
        eps_ap = _c(tc, con, "eps_ap", (128, 1))
        zcol = _c(tc, con, "zcol", (128, 1))
        maskA = _c(tc, con, "maskA", (SL, NSEQ))

        WqT = _c(tc, con, "WqT", (POS, POS))
        WkT = _c(tc, con, "WkT", (POS, POS))
        WvT = _c(tc, con, "WvT", (POS, POS))
        WoT = _c(tc, con, "WoT", (POS, POS))
        bq_rep = _c(tc, con, "bq_rep", (128, POS))
        bk_rep = _c(tc, con, "bk_rep", (128, POS))
        bv_rep = _c(tc, con, "bv_rep", (128, POS))
        bo_rep = _c(tc, con, "bo_rep", (128, POS))
        WvcaT = _c(tc, con, "WvcaT", (POS, POS))
        bvca = _c(tc, con, "bvca", (POS, 1))
        WocaT = _c(tc, con, "WocaT", (POS, POS))
        cab_rep = _c(tc, con, "cab_rep", (128, POS))
        g1_rep = _c(tc, con, "g1_rep", (128, POS))
        W1T = _c(tc, con, "W1T", (POS, FF))
        b1ff = _c(tc, con, "b1ff", (FF, 1))
        g2_rep = _c(tc, con, "g2_rep", (128, POS))
        W2T = _c(tc, con, "W2T", (FF, POS))
        b2_rep = _c(tc, con, "b2_rep", (128, POS))
        g3_rep = _c(tc, con, "g3_rep", (128, POS))
        b3_rep = _c(tc, con, "b3_rep", (128, POS))
        gon_rep = _c(tc, con, "gon_rep", (128, POS))
        bon_rep = _c(tc, con, "bon_rep", (128, POS))

        WeqT = _c(tc, con, "WeqT", (POS, POS))
        WekT = _c(tc, con, "WekT", (POS, POS))
        WevT = _c(tc, con, "WevT", (POS, POS))
        WeoT = _c(tc, con, "WeoT", (POS, POS))
        beq = _c(tc, con, "beq", (POS, 1))
        bek = _c(tc, con, "bek", (POS, 1))
        bev_rep = _c(tc, con, "bev_rep", (128, POS))
        beo = _c(tc, con, "beo", (POS, 1))
        We1T = _c(tc, con, "We1T", (POS, POS))
        be1ff = _c(tc, con, "be1ff", (POS, 1))
        We2T = _c(tc, con, "We2T", (POS, POS))
        be2ff = _c(tc, con, "be2ff", (POS, 1))

        ctx_glob = tc.tile_pool(name="glob", bufs=1)
        glob = ctx_glob.__enter__()
        topoT = [glob.tile([128, NCOL], F32, name=f"topoT{d}") for d in range(2)]

        def cols(tt, t):
            return tt * (T * SP) + t * SP

        def mm(out, lhsT, rhs, start, stop):
            PE.matmul(out, lhsT, rhs, start=start, stop=stop)

        def layernorm(pool, src, name):
            bst = pool.tile([128, 6], F32, name=name + "_bst", tag="ln_bst")
            bag = pool.tile([128, 2], F32, name=name + "_bag", tag="ln_bag")
            sd = pool.tile([128, 1], F32, name=name + "_sd", tag="ln_sd")
            inv = pool.tile([128, 1], F32, name=name + "_inv", tag="ln_inv")
            z = pool.tile([128, POS], F32, name=name + "_z", tag=name + "_z", bufs=1)
            V.bn_stats(bst[:, :], src)
            V.bn_aggr(bag[:, :], bst[:, :])
            A.activation(sd[:, :], bag[:, 1:2], AF.Sqrt, bias=eps_ap[:, 0:1])
            V.reciprocal(inv[:, :], sd[:, :])
            V.tensor_scalar(z[:, :], src, bag[:, 0:1], inv[:, :],
                            OP.subtract, OP.mult)
            return z

        # ---------------- prologue ----------------
        mem_tok = [glob.tile([128, POS], F32, name=f"memtok{tt}")
                   for tt in range(2)]
        ca_sb = [glob.tile([128, POS], F32, name=f"ca{tt}") for tt in range(2)]
        with tc.tile_pool(name="pro", bufs=2) as pro, \
                tc.tile_pool(name="prop", bufs=2, space="PSUM") as prop:
            for tt in range(2):
                gb = prop.tile([128, 2 * POS], F32, name="gb", tag="pp")
                mm(gb[:, :], fT[:, tt * 128:(tt + 1) * 128], se_wT[:, :],
                   True, True)
                tg = pro.tile([128, POS], F32, name="tg", tag="tg")
                tb = pro.tile([128, POS], F32, name="tb", tag="tb")
                V.tensor_tensor(tg[:, :], gb[:, :POS], se_bg[:, :], OP.add)
                V.tensor_tensor(tg[:, :], tg[:, :], xe[tt][:, :], OP.mult)
                V.tensor_tensor(tb[:, :], gb[:, POS:], se_bb[:, :], OP.add)
                V.tensor_tensor(tg[:, :], tg[:, :], tb[:, :], OP.add)
                sh = pro.tile([128, POS], F32, name="sh", tag="sh")
                A.activation(sh[:, :], tg[:, :], AF.Tanh)
                td = prop.tile([128, POS], F32, name="td", tag="pp")
                mm(td[:, :], cosC[:, :], sh[:, :], True, True)
                A.activation(mem_tok[tt][:, :], td[:, :], AF.Identity,
                             bias=zcol[:, 0:1])
                for d in range(2):
                    tp = prop.tile([128, 128], F32, name="tp", tag="pp")
                    PE.transpose(tp[:, :], mem_tok[tt][:, d * 128:(d + 1) * 128],
                                 ident[:, :])
                    c0 = cols(tt, 0)
                    A.activation(topoT[d][:, c0:c0 + SL], tp[:, :], AF.Identity,
                                 bias=zcol[:, 0:1])
            for d in range(2):
                dst = topoT[d][:, :].rearrange("p (b t a) -> p b t a",
                                               b=BL, t=T, a=SP)[:, :, :, SL:SP]
                nc.sync.dma_start(
                    dst, goal_d[d * 128:(d + 1) * 128, :, :].unsqueeze(3))
            vmem = []
            for d in range(2):
                vp = prop.tile([128, POS], F32, name="vp", tag="pp")
                for b in range(2):
                    for dj in range(2):
                        mm(vp[:, b * 128:(b + 1) * 128],
                           WvcaT[dj][:, d * 128:(d + 1) * 128],
                           topoT[dj][:, cols(b, 0):cols(b, 0) + SL],
                           dj == 0, dj == 1)
                vm = pro.tile([128, POS], F32, name="vm", tag=f"vm{d}")
                A.activation(vm[:, :], vp[:, :], AF.Identity,
                             bias=bvca[d][:, 0:1])
                vmem.append(vm)
            for tt in range(2):
                cp = prop.tile([128, POS], F32, name="cp", tag="pp")
                for d in range(2):
                    mm(cp[:, :], vmem[d][:, tt * 128:(tt + 1) * 128],
                       WocaT[d][:, :], d == 0, d == 1)
                V.tensor_tensor(ca_sb[tt][:, :], cp[:, :], cab_rep[:, :], OP.add)

        # ---------------- decoder ----------------
        with tc.tile_pool(name="decg", bufs=1) as decg, \
                tc.tile_pool(name="dec", bufs=2) as dec, \
                tc.tile_pool(name="decp", bufs=2, space="PSUM") as decp:
            kcache = [decg.tile([128, (T - 1) * POS], F32, name=f"kc{tt}")
                      for tt in range(2)]
            vcache = [decg.tile([128, (T - 1) * POS], F32, name=f"vc{tt}")
                      for tt in range(2)]
            oT = [decg.tile([128, POS], F32, name=f"oT{d}") for d in range(2)]
            z2T = [decg.tile([128, POS], F32, name=f"z2T{d}") for d in range(2)]
            r_prev = list(mem_tok)
            for i in range(1, T):
                z2s = []
                for tt in range(2):
                    xc = cols(tt, i - 1)
                    psq = decp.tile([128, POS], F32, name="psq", tag="qkv")
                    psk = decp.tile([128, POS], F32, name="psk", tag="qkv")
                    psv = decp.tile([128, POS], F32, name="psv", tag="qkv")
                    for d in range(2):
                        lx = topoT[d][:, xc:xc + SL]
                        mm(psq[:, :], lx, WqT[d][:, :], d == 0, d == 1)
                        mm(psk[:, :], lx, WkT[d][:, :], d == 0, d == 1)
                        mm(psv[:, :], lx, WvT[d][:, :], d == 0, d == 1)
                    q_sb = dec.tile([128, POS], F32, name="q_sb", tag="q_sb", bufs=1)
                    V.tensor_tensor(q_sb[:, :], psq[:, :], bq_rep[:, :], OP.add)
                    ks = kcache[tt][:, (i - 1) * POS:i * POS]
                    vs = vcache[tt][:, (i - 1) * POS:i * POS]
                    V.tensor_tensor(ks, psk[:, :], bk_rep[:, :], OP.add)
                    V.tensor_tensor(vs, psv[:, :], bv_rep[:, :], OP.add)

                    prod = dec.tile([128, T * POS], F32, name="prod", tag="pq", bufs=1)
                    qb = q_sb[:, :].unsqueeze(1).broadcast_to((128, i, POS))
                    V.tensor_tensor(
                        prod[:, :i * POS].rearrange("p (j f) -> p j f", j=i),
                        kcache[tt][:, :i * POS].rearrange("p (j f) -> p j f",
                                                          j=i),
                        qb, OP.mult)
                    sc = dec.tile([128, T * H], F32, name="sc", tag="sc")
                    V.tensor_reduce(
                        sc[:, :i * H],
                        prod[:, :i * POS].rearrange("p (g w) -> p g w", w=DH),
                        AX.X, OP.add)
                    pe_ = dec.tile([128, T * H], F32, name="pe_", tag="pe_")
                    A.activation(pe_[:, :i * H], sc[:, :i * H], AF.Exp)
                    den = dec.tile([128, H], F32, name="den", tag="den")
                    V.tensor_reduce(
                        den[:, :],
                        pe_[:, :i * H].rearrange("p (j h) -> p h j", h=H),
                        AX.X, OP.add)
                    rec = dec.tile([128, H], F32, name="rec", tag="rec")
                    V.reciprocal(rec[:, :], den[:, :])
                    pn = dec.tile([128, T * H], F32, name="pn", tag="pn")
                    rb = rec[:, :].unsqueeze(1).broadcast_to((128, i, H))
                    V.tensor_tensor(
                        pn[:, :i * H].rearrange("p (j h) -> p j h", h=H),
                        pe_[:, :i * H].rearrange("p (j h) -> p j h", h=H),
                        rb, OP.mult)
                    pv = dec.tile([128, T * POS], F32, name="pv", tag="pq", bufs=1)
                    pnb = pn[:, :i * H].rearrange("p (j h) -> p h j", h=H) \
                        .unsqueeze(2).broadcast_to((128, H, DH, i))
                    G.tensor_tensor(
                        pv[:, :i * POS].rearrange("p (h e j) -> p h e j",
                                                  h=H, e=DH),
                        vcache[tt][:, :i * POS].rearrange(
                            "p (j h e) -> p h e j", h=H, e=DH),
                        pnb, OP.mult)
                    o_sb = dec.tile([128, POS], F32, name="o_sb", tag="o_sb")
                    V.tensor_reduce(
                        o_sb[:, :],
                        pv[:, :i * POS].rearrange("p (f j) -> p f j", j=i),
                        AX.X, OP.add)
                    for d in range(2):
                        tp = decp.tile([128, 128], F32, name="tp2", tag="A")
                        PE.transpose(tp[:, :], o_sb[:, d * 128:(d + 1) * 128],
                                     ident[:, :])
                        A.activation(oT[d][:, tt * 128:(tt + 1) * 128],
                                     tp[:, :], AF.Identity, bias=zcol[:, 0:1])
                    sa = decp.tile([128, POS], F32, name="sa", tag="A")
                    for d in range(2):
                        mm(sa[:, :], oT[d][:, tt * 128:(tt + 1) * 128],
                           WoT[d][:, :], d == 0, False)
                    mm(sa[:, :], ident[:, :], bo_rep[:, :], False, False)
                    mm(sa[:, :], ident[:, :], r_prev[tt][:, :], False, True)
                    z1 = layernorm(dec, sa[:, :], "ln1")
                    t2 = dec.tile([128, POS], F32, name="t2", tag="t2", bufs=1)
                    V.tensor_tensor(t2[:, :], z1[:, :], g1_rep[:, :], OP.mult)
                    V.tensor_tensor(t2[:, :], t2[:, :], ca_sb[tt][:, :], OP.add)
                    z2 = layernorm(dec, t2[:, :], f"ln2_{tt}")
                    z2s.append(z2)
                    for d in range(2):
                        tp = decp.tile([128, 128], F32, name="tp3", tag="A")
                        PE.transpose(tp[:, :], z2[:, d * 128:(d + 1) * 128],
                                     ident[:, :])
                        A.activation(z2T[d][:, tt * 128:(tt + 1) * 128],
                                     tp[:, :], AF.Identity, bias=zcol[:, 0:1])
                ff1r = []
                for f in range(8):
                    fp = decp.tile([128, POS], F32, name="fp", tag="A")
                    for d in range(2):
                        mm(fp[:, :], W1T[d][:, f * 128:(f + 1) * 128],
                           z2T[d][:, :], d == 0, d == 1)
                    fr = dec.tile([128, POS], F32, name="fr", tag=f"fr{f}", bufs=1)
                    A.activation(fr[:, :], fp[:, :], AF.Relu,
                                 bias=b1ff[f][:, 0:1])
                    ff1r.append(fr)
                for tt in range(2):
                    f2 = decp.tile([128, POS], F32, name="f2", tag="A")
                    for f in range(8):
                        mm(f2[:, :], ff1r[f][:, tt * 128:(tt + 1) * 128],
                           W2T[f][:, :], f == 0, False)
                    mm(f2[:, :], ident[:, :], b2_rep[:, :], False, True)
                    t3 = dec.tile([128, POS], F32, name="t3", tag="t3", bufs=1)
                    V.tensor_tensor(t3[:, :], z2s[tt][:, :], g2_rep[:, :],
                                    OP.mult)
                    V.tensor_tensor(t3[:, :], t3[:, :], f2[:, :], OP.add)
                    z3 = layernorm(dec, t3[:, :], "ln3")
                    x3 = dec.tile([128, POS], F32, name="x3", tag="x3", bufs=1)
                    V.tensor_tensor(x3[:, :], z3[:, :], g3_rep[:, :], OP.mult)
                    V.tensor_tensor(x3[:, :], x3[:, :], b3_rep[:, :], OP.add)
                    zon = layernorm(dec, x3[:, :], "lnon")
                    rnew = dec.tile([128, POS], F32, name="rnew", tag=f"rn{tt}")
                    V.tensor_tensor(rnew[:, :], zon[:, :], gon_rep[:, :],
                                    OP.mult)
                    V.tensor_tensor(rnew[:, :], rnew[:, :], bon_rep[:, :],
                                    OP.add)
                    r_prev[tt] = rnew
                    for d in range(2):
                        tp = decp.tile([128, 128], F32, name="tp4", tag="A")
                        PE.transpose(tp[:, :], rnew[:, d * 128:(d + 1) * 128],
                                     ident[:, :])
                        c0 = cols(tt, i)
                        A.activation(topoT[d][:, c0:c0 + SL], tp[:, :],
                                     AF.Identity, bias=zcol[:, 0:1])

        # ---------------- encoder ----------------
        with tc.tile_pool(name="encg", bufs=1) as encg, \
                tc.tile_pool(name="enc", bufs=2) as enc, \
                tc.tile_pool(name="encp", bufs=2, space="PSUM") as encp:
            stats1 = encg.tile([NSEQ, 2 * SP], F32, name="stats1")
            stats2 = encg.tile([NSEQ, 2 * SP], F32, name="stats2")
            xenc = [[None, None] for _ in range(NSEQ)]

            def seq_ln_stats(src0, src1, stats, s):
                sp_ = encp.tile([1, 2 * SP], F32, name="sp_", tag="sml")
                sq0 = enc.tile([128, SP], F32, name="sq0", tag="sq0")
                sq1 = enc.tile([128, SP], F32, name="sq1", tag="sq1")
                A.activation(sq0[:, :], src0, AF.Square)
                A.activation(sq1[:, :], src1, AF.Square)
                mm(sp_[:, 0:SP], ones128[:, :], src0, True, False)
                mm(sp_[:, 0:SP], ones128[:, :], src1, False, True)
                mm(sp_[:, SP:], ones128[:, :], sq0[:, :], True, False)
                mm(sp_[:, SP:], ones128[:, :], sq1[:, :], False, True)
                stg = enc.tile([1, 2 * SP], F32, name="stg", tag="stg")
                A.activation(stg[:, :], sp_[:, :], AF.Identity,
                             bias=zcol[0:1, 0:1])
                nc.sync.dma_start(stats[s:s + 1, :], stg[:, :])

            def stats_to_minv(stats, mtile, itile):
                V.tensor_scalar_mul(mtile[:, :], stats[:, 0:SP], 1.0 / POS)
                msq = enc.tile([NSEQ, SP], F32, name="msq", tag="msq")
                V.tensor_tensor(msq[:, :], mtile[:, :], mtile[:, :], OP.mult)
                var = enc.tile([NSEQ, SP], F32, name="var", tag="var")
                V.scalar_tensor_tensor(var[:, :], stats[:, SP:], 1.0 / POS,
                                       msq[:, :], OP.mult, OP.subtract)
                sdv = enc.tile([NSEQ, SP], F32, name="sdv", tag="sdv")
                A.activation(sdv[:, :], var[:, :], AF.Sqrt,
                             bias=eps_ap[:NSEQ, 0:1])
                V.reciprocal(itile[:, :], sdv[:, :])

            for s in range(NSEQ):
                b, t = divmod(s, T)
                c0 = cols(b, t)
                seq_ln_stats(topoT[0][:, c0:c0 + SP], topoT[1][:, c0:c0 + SP],
                             stats1, s)
            m1 = encg.tile([NSEQ, SP], F32, name="m1")
            i1 = encg.tile([NSEQ, SP], F32, name="i1")
            stats_to_minv(stats1, m1, i1)

            for s in range(NSEQ):
                b, t = divmod(s, T)
                c0 = cols(b, t)
                mi = enc.tile([1, 2 * SP], F32, name="mi", tag="mi")
                nc.sync.dma_start(mi[:, 0:SP], m1[s:s + 1, :])
                nc.sync.dma_start(mi[:, SP:], i1[s:s + 1, :])
                rep = encp.tile([128, 2 * SP], F32, name="rep", tag="sml")
                mm(rep[:, 0:SP], onesr[:, :], mi[:, 0:SP], True, True)
                mm(rep[:, SP:], onesr[:, :], mi[:, SP:], True, True)
                hs = []
                for d in range(2):
                    h = enc.tile([128, SP], F32, name="h", tag=f"h{d}")
                    V.tensor_tensor(h[:, :], topoT[d][:, c0:c0 + SP],
                                    rep[:, 0:SP], OP.subtract)
                    V.tensor_tensor(h[:, :], h[:, :], rep[:, SP:], OP.mult)
                    hs.append(h)
                qT, kT = [], []
                for d in range(2):
                    qp = encp.tile([128, SP], F32, name="qp", tag="sml")
                    kp = encp.tile([128, SP], F32, name="kp", tag="sml")
                    for dj in range(2):
                        mm(qp[:, :], WeqT[dj][:, d * 128:(d + 1) * 128],
                           hs[dj][:, :], dj == 0, dj == 1)
                        mm(kp[:, :], WekT[dj][:, d * 128:(d + 1) * 128],
                           hs[dj][:, :], dj == 0, dj == 1)
                    qs = enc.tile([128, SP], F32, name="qs", tag=f"qs{d}")
                    ks = enc.tile([128, SP], F32, name="ks", tag=f"ks{d}")
                    A.activation(qs[:, :], qp[:, :], AF.Identity,
                                 bias=beq[d][:, 0:1])
                    A.activation(ks[:, :], kp[:, :], AF.Identity,
                                 bias=bek[d][:, 0:1])
                    qT.append(qs)
                    kT.append(ks)
                vAp = encp.tile([128, POS], F32, name="vAp", tag="sml")
                vGp = encp.tile([1, POS], F32, name="vGp", tag="sml")
                for dj in range(2):
                    mm(vAp[:, :], hs[dj][:, 0:SL], WevT[dj][:, :],
                       dj == 0, dj == 1)
                    mm(vGp[:, :], hs[dj][:, SL:SP], WevT[dj][:, :],
                       dj == 0, dj == 1)
                vA = enc.tile([128, POS], F32, name="vA", tag="vA")
                vG = enc.tile([1, POS], F32, name="vG", tag="vG")
                V.tensor_tensor(vA[:, :], vAp[:, :], bev_rep[:, :], OP.add)
                V.tensor_tensor(vG[:, :], vGp[:, :], bev_rep[0:1, :], OP.add)
                scp = encp.tile([128, 1024], F32, name="scp", tag="big", bufs=1)
                scg = encp.tile([1, 1024], F32, name="scg", tag="scg", bufs=1)
                for h in range(EH):
                    d, r0 = divmod(h * EDH, 128)
                    lk = kT[d][r0:r0 + EDH, :]
                    lq = qT[d][r0:r0 + EDH, :]
                    mm(scp[:, h * 256:h * 256 + SP], lk[:, 0:SL], lq,
                       True, True)
                    mm(scg[:, h * 256:h * 256 + SP], lk[:, SL:SP], lq,
                       True, True)
                probs = enc.tile([128, EH * SP], F32, name="probs", tag="probs")
                pg = enc.tile([1, EH * SP], F32, name="pg", tag="pg")
                A.activation(probs[:, :].rearrange("p (h q) -> p h q", q=SP),
                             scp[:, :].rearrange("p (h q) -> p h q", q=256)
                             [:, :, 0:SP], AF.Exp, bias=maskA[:, s:s + 1])
                A.activation(pg[:, :].rearrange("p (h q) -> p h q", q=SP),
                             scg[:, :].rearrange("p (h q) -> p h q", q=256)
                             [:, :, 0:SP], AF.Exp, bias=zcol[0:1, 0:1])
                dn = encp.tile([1, 2 * SP], F32, name="dn", tag="sml")
                dn2 = encp.tile([1, 2 * SP], F32, name="dn2", tag="sml")
                rraw = enc.tile([1, EH * SP], F32, name="rraw", tag="rraw")
                for p, dtile in enumerate((dn, dn2)):
                    mm(dtile[:, :], ones128[:, :],
                       probs[:, p * 2 * SP:(p + 1) * 2 * SP], True, False)
                    mm(dtile[:, :], ones128[0:1, 0:1],
                       pg[:, p * 2 * SP:(p + 1) * 2 * SP], False, True)
                    A.activation(rraw[:, p * 2 * SP:(p + 1) * 2 * SP],
                                 dtile[:, :], AF.Identity, bias=zcol[0:1, 0:1])
                rcb = enc.tile([1, EH * SP], F32, name="rcb", tag="rcb")
                V.reciprocal(rcb[:, :], rraw[:, :])
                rp = encp.tile([128, 1024], F32, name="rp", tag="big", bufs=1)
                for h in range(EH):
                    mm(rp[:, h * 256:h * 256 + SP], onesr[:, :],
                       rcb[:, h * SP:(h + 1) * SP], True, True)
                psc = enc.tile([128, EH * SP], F32, name="psc", tag="psc")
                V.tensor_tensor(psc[:, :].rearrange("p (h q) -> p h q", q=SP),
                                probs[:, :].rearrange("p (h q) -> p h q", q=SP),
                                rp[:, :].rearrange("p (h q) -> p h q", q=256)
                                [:, :, 0:SP], OP.mult)
                pgs = enc.tile([1, EH * SP], F32, name="pgs", tag="pgs")
                V.tensor_tensor(pgs[:, :], pg[:, :], rcb[:, :], OP.mult)
                oTs = []
                for p in range(2):
                    op_ = encp.tile([128, SP], F32, name="op_", tag="sml")
                    for hh in range(2):
                        h = 2 * p + hh
                        PE.matmul(op_[hh * EDH:(hh + 1) * EDH, :],
                                  vA[:, h * EDH:(h + 1) * EDH],
                                  psc[:, h * SP:(h + 1) * SP],
                                  start=True, stop=False,
                                  tile_position=(0, hh * EDH))
                        PE.matmul(op_[hh * EDH:(hh + 1) * EDH, :],
                                  vG[:, h * EDH:(h + 1) * EDH],
                                  pgs[:, h * SP:(h + 1) * SP],
                                  start=False, stop=True,
                                  tile_position=(0, hh * EDH))
                    os_ = enc.tile([128, SP], F32, name="os_", tag=f"os{p}")
                    A.activation(os_[:, :], op_[:, :], AF.Identity,
                                 bias=zcol[:, 0:1])
                    oTs.append(os_)
                for d in range(2):
                    yp = encp.tile([128, SP], F32, name="yp", tag="sml")
                    for kt in range(2):
                        mm(yp[:, :], WeoT[kt][:, d * 128:(d + 1) * 128],
                           oTs[kt][:, :], kt == 0, kt == 1)
                    xe_ = encg.tile([128, SP], F32, name=f"xenc{s}_{d}")
                    V.scalar_tensor_tensor(xe_[:, :], yp[:, :], beo[d][:, 0:1],
                                           topoT[d][:, c0:c0 + SP],
                                           OP.add, OP.add)
                    xenc[s][d] = xe_
                seq_ln_stats(xenc[s][0][:, :], xenc[s][1][:, :], stats2, s)

            m2 = encg.tile([NSEQ, SP], F32, name="m2")
            i2 = encg.tile([NSEQ, SP], F32, name="i2")
            stats_to_minv(stats2, m2, i2)

            for s in range(NSEQ):
                b, t = divmod(s, T)
                c0 = cols(b, t)
                mi2 = enc.tile([1, 2 * SP], F32, name="mi2", tag="mi2")
                nc.sync.dma_start(mi2[:, 0:SP], m2[s:s + 1, :])
                nc.sync.dma_start(mi2[:, SP:], i2[s:s + 1, :])
                rep = encp.tile([128, 2 * SP], F32, name="rep2", tag="sml")
                mm(rep[:, 0:SP], onesr[:, :], mi2[:, 0:SP], True, True)
                mm(rep[:, SP:], onesr[:, :], mi2[:, SP:], True, True)
                h2 = []
                for d in range(2):
                    h = enc.tile([128, SP], F32, name="h2", tag=f"h2{d}")
                    V.tensor_tensor(h[:, :], xenc[s][d][:, :], rep[:, 0:SP],
                                    OP.subtract)
                    V.tensor_tensor(h[:, :], h[:, :], rep[:, SP:], OP.mult)
                    h2.append(h)
                f1r = []
                for d in range(2):
                    f1p = encp.tile([128, SP], F32, name="f1p", tag="sml")
                    for dj in range(2):
                        mm(f1p[:, :], We1T[dj][:, d * 128:(d + 1) * 128],
                           h2[dj][:, :], dj == 0, dj == 1)
                    fr = enc.tile([128, SP], F32, name="fr2", tag=f"fr2{d}")
                    A.activation(fr[:, :], f1p[:, :], AF.Relu,
                                 bias=be1ff[d][:, 0:1])
                    f1r.append(fr)
                for d in range(2):
                    y2 = encp.tile([128, SP], F32, name="y2", tag="sml")
                    for dj in range(2):
                        mm(y2[:, :], We2T[dj][:, d * 128:(d + 1) * 128],
                           f1r[dj][:, :], dj == 0, dj == 1)
                    of = enc.tile([128, SP], F32, name="of", tag=f"of{d}")
                    V.scalar_tensor_tensor(of[:, :], y2[:, :], be2ff[d][:, 0:1],
                                           xenc[s][d][:, :], OP.add, OP.add)
                    nc.sync.dma_start(out_d[d * 128:(d + 1) * 128, c0:c0 + SP],
                                      of[:, :])
        ctx_glob.__exit__(None, None, None)
        ctx_con.__exit__(None, None, None)
    nc.compile()
    return nc


def _rep(v):
    return np.tile(np.asarray(v, np.float32)[None, :], (128, 1))


def _prep_shared(params):
    p = {k: np.asarray(v, np.float32) for k, v in params.items()}
    sc = 1.0 / math.sqrt(DH)
    wq, wk, wv = np.split(p["d_sa_wi"], 3, 0)
    bq, bk, bv = np.split(p["d_sa_bi"], 3, 0)
    n, k = np.meshgrid(np.arange(SL), np.arange(SL), indexing="ij")
    cosC = (np.cos(2 * np.pi * n * k / SL) / SL).astype(np.float32)
    esc = 1.0 / math.sqrt(EDH)
    ewq, ewk, ewv = np.split(p["e_sa_wi"], 3, 0)
    ebq, ebk, ebv = np.split(p["e_sa_bi"], 3, 0)
    ge1, be1 = p["e_n1_g"], p["e_n1_b"]
    ewq_f, ewk_f, ewv_f = ewq * ge1, ewk * ge1, ewv * ge1
    ebq_f = ebq + ewq @ be1
    ebk_f = ebk + ewk @ be1
    ebv_f = ebv + ewv @ be1
    ge2, be2 = p["e_n2_g"], p["e_n2_b"]
    we1_f = p["e_l1_w"] * ge2[None, :]
    be1ff = p["e_l1_b"] + p["e_l1_w"] @ be2
    wvca = p["d_ca_wi"][2 * POS:3 * POS]
    bvca = p["d_ca_bi"][2 * POS:3 * POS]
    d = {
        "se_wT": p["se_w"].T, "se_bg": _rep(p["se_b"][:POS]),
        "se_bb": _rep(p["se_b"][POS:]), "cosC": cosC,
        "ident": np.eye(128, dtype=np.float32),
        "ones128": np.ones((128, 1), np.float32),
        "onesr": np.ones((1, 128), np.float32),
        "eps_ap": np.full((128, 1), EPS, np.float32),
        "zcol": np.zeros((128, 1), np.float32),
        "WqT": wq.T * sc, "WkT": wk.T, "WvT": wv.T, "WoT": p["d_sa_wo"].T,
        "bq_rep": _rep(bq * sc), "bk_rep": _rep(bk), "bv_rep": _rep(bv),
        "bo_rep": _rep(p["d_sa_bo"]),
        "WvcaT": wvca.T, "bvca": bvca[:, None],
        "WocaT": p["d_ca_wo"].T,
        "cab_rep": _rep(p["d_ca_bo"] + p["d_n1_b"]),
        "g1_rep": _rep(p["d_n1_g"]),
        "W1T": (p["d_l1_w"] * p["d_n2_g"][None, :]).T,
        "b1ff": (p["d_l1_b"] + p["d_l1_w"] @ p["d_n2_b"])[:, None],
        "g2_rep": _rep(p["d_n2_g"]), "W2T": p["d_l2_w"].T,
        "b2_rep": _rep(p["d_l2_b"] + p["d_n2_b"]),
        "g3_rep": _rep(p["d_n3_g"]), "b3_rep": _rep(p["d_n3_b"]),
        "gon_rep": _rep(p["on_g"]), "bon_rep": _rep(p["on_b"]),
        "WeqT": ewq_f.T * esc, "WekT": ewk_f.T, "WevT": ewv_f.T,
        "WeoT": p["e_sa_wo"].T,
        "beq": (ebq_f * esc)[:, None], "bek": ebk_f[:, None],
        "bev_rep": _rep(ebv_f), "beo": p["e_sa_bo"][:, None],
        "We1T": we1_f.T, "be1ff": be1ff[:, None],
        "We2T": p["e_l2_w"].T, "be2ff": p["e_l2_b"][:, None],
    }
    return {k: np.ascontiguousarray(v, np.float32) for k, v in d.items()}


def kernel(agent_emb, agent_feature, goal_emb, agent_mask, params):
    agent_emb = np.asarray(agent_emb, np.float32)
    agent_feature = np.asarray(agent_feature, np.float32)
    goal_emb = np.asarray(goal_emb, np.float32)
    agent_mask = np.asarray(agent_mask)
    if "nc" not in _CACHE:
        _CACHE["nc"] = build_kernel()
    nc = _CACHE["nc"]
    shared = _prep_shared(params)
    in_maps = []
    for c in range(NCORES):
        b0 = BL * c
        vals = dict(shared)
        vals["xe_tok"] = agent_emb[b0:b0 + BL].reshape(S, POS)
        vals["fT"] = agent_feature[b0:b0 + BL].reshape(S, FEAT).T
        km = np.where(agent_mask[b0:b0 + BL], 0.0, NEG).astype(np.float32)
        vals["maskA"] = km.transpose(2, 0, 1).reshape(SL, NSEQ)
        m = {
            "cpack": _build_pack(vals),
            "goalT_bc": np.ascontiguousarray(
                np.repeat(goal_emb[b0:b0 + BL].T[:, :, None], T, axis=2)),
        }
        in_maps.append(m)
    res = run_bass_kernel_spmd(nc, in_maps, core_ids=list(range(NCORES)))
    outs = []
    for c in range(NCORES):
        o = np.asarray(res.results[c]["out_T"])
        outs.append(o.T.reshape(BL, T, SP, POS))
    return np.concatenate(outs, 0).astype(np.float32)


if __name__ == "__main__":
    build_kernel()
    print("build ok")


# revision 20
# speedup vs baseline: 1.0318x; 1.0318x over previous
"""Trainium2 Bass kernel for nn_BackgroundEncoder.

Data-parallel over bz across 8 cores (2 samples/core). Inside each core:
  FiLM -> iFFT(cos matmul) -> 15-step KV-cached decoder -> masked encoder layer.
All math fp32. Host pre-transposes/folds weights and gathers output.
"""
import math
import sys

import numpy as np

sys.path.insert(0, "/opt/trn_rl_repo")

import concourse.bass as bass
import concourse.bacc as bacc
import concourse.mybir as mybir
from concourse.bass_utils import run_bass_kernel_spmd
from concourse.tile import TileContext

F32 = mybir.dt.float32
AX = mybir.AxisListType
OP = mybir.AluOpType
AF = mybir.ActivationFunctionType

POS = 256
FEAT = 7
BZ = 16
SL = 128
T = 16
H = 8
DH = 32
FF = 1024
EH = 4
EDH = 64
NCORES = 8
BL = 2
S = BL * SL
NSEQ = BL * T
SP = SL + 1
NCOL = NSEQ * SP
NEG = -30000.0
EPS = 1e-5

_CACHE = {}


_PSPEC = [
    ("xe_tok", (S, POS)), ("fT", (FEAT, S)), ("se_wT", (FEAT, 2 * POS)),
    ("se_bg", (128, POS)), ("se_bb", (128, POS)), ("cosC", (SL, SL)),
    ("ident", (128, 128)), ("ones128", (128, 1)), ("onesr", (1, 128)),
    ("eps_ap", (128, 1)), ("zcol", (128, 1)), ("maskA", (SL, NSEQ)),
    ("WqT", (POS, POS)), ("WkT", (POS, POS)), ("WvT", (POS, POS)),
    ("WoT", (POS, POS)), ("bq_rep", (128, POS)), ("bk_rep", (128, POS)),
    ("bv_rep", (128, POS)), ("bo_rep", (128, POS)), ("WvcaT", (POS, POS)),
    ("bvca", (POS, 1)), ("WocaT", (POS, POS)), ("cab_rep", (128, POS)),
    ("g1_rep", (128, POS)), ("W1T", (POS, FF)), ("b1ff", (FF, 1)),
    ("g2_rep", (128, POS)), ("W2T", (FF, POS)), ("b2_rep", (128, POS)),
    ("g3_rep", (128, POS)), ("b3_rep", (128, POS)), ("gon_rep", (128, POS)),
    ("bon_rep", (128, POS)), ("WeqT", (POS, POS)), ("WekT", (POS, POS)),
    ("WevT", (POS, POS)), ("WeoT", (POS, POS)), ("beq", (POS, 1)),
    ("bek", (POS, 1)), ("bev_rep", (128, POS)), ("beo", (POS, 1)),
    ("We1T", (POS, POS)), ("be1ff", (POS, 1)), ("We2T", (POS, POS)),
    ("be2ff", (POS, 1)),
]


def _build_pack(vals):
    lay, total = _playout()
    a = np.zeros((128, total), np.float32)
    for name, (r, c) in _PSPEC:
        col, rr, cc, n = lay[name]
        v = np.asarray(vals[name], np.float32)
        if n == 1:
            a[0:r, col:col + c] = v
        else:
            for i in range(n):
                a[:, col + i * c:col + (i + 1) * c] = v[i * 128:(i + 1) * 128]
    return a


def _playout():
    lay, cur = {}, 0
    for name, (r, c) in _PSPEC:
        n = 1 if r <= 128 else r // 128
        lay[name] = (cur, r, c, n)
        cur += c * n
    return lay, cur


class _Pack:
    def __init__(self):
        self.lay, self.total = _playout()
        self.tile = None

    def get(self, name):
        col, r, c, n = self.lay[name]
        if n == 1:
            return self.tile[0:min(r, 128), col:col + c]
        return [self.tile[:, col + i * c:col + (i + 1) * c] for i in range(n)]


def _c(tc, pool, name, shape):
    return tc._pack.get(name)


def build_kernel():
    nc = bacc.Bacc("TRN2", target_bir_lowering=False)
    out_d = nc.dram_tensor("out_T", [POS, NCOL], F32, kind="ExternalOutput")
    goal_d = nc.dram_tensor("goalT_bc", [POS, BL, T], F32, kind="ExternalInput")

    with TileContext(nc) as tc:
        V = nc.vector
        G = nc.gpsimd
        A = nc.scalar
        PE = nc.tensor

        ctx_con = tc.tile_pool(name="con", bufs=1)
        con = ctx_con.__enter__()
        pack = _Pack()
        tc._pack = pack
        cpack_d = nc.dram_tensor("cpack", [128, pack.total], F32,
                                 kind="ExternalInput")
        cpack = con.tile([128, pack.total], F32, name="cpack")
        nc.gpsimd.dma_start(cpack[:, :], cpack_d[:, :])
        pack.tile = cpack
        warm = con.tile([128, 3], F32, name="warm")
        nc.vector.tensor_copy(warm[:, 0:1], cpack[:, 0:1])
        nc.scalar.activation(warm[:, 1:2], cpack[:, 0:1], AF.Identity,
                             bias=cpack[:, 0:1])
        nc.gpsimd.tensor_copy(warm[:, 2:3], cpack[:, 0:1])
        xe = _c(tc, con, "xe_tok", (S, POS))
        fT = _c(tc, con, "fT", (FEAT, S))
        se_wT = _c(tc, con, "se_wT", (FEAT, 2 * POS))
        se_bg = _c(tc, con, "se_bg", (128, POS))
        se_bb = _c(tc, con, "se_bb", (128, POS))
        cosC = _c(tc, con, "cosC", (SL, SL))
        ident = _c(tc, con, "ident", (128, 128))
        ones128 = _c(tc, con, "ones128", (128, 1))
        onesr = _c(tc, con, "onesr", (1, 128))
        eps_ap = _c(tc, con, "eps_ap", (128, 1))
        zcol = _c(tc, con, "zcol", (128, 1))
        maskA = _c(tc, con, "maskA", (SL, NSEQ))

        WqT = _c(tc, con, "WqT", (POS, POS))
        WkT = _c(tc, con, "WkT", (POS, POS))
        WvT = _c(tc, con, "WvT", (POS, POS))
        WoT = _c(tc, con, "WoT", (POS, POS))
        bq_rep = _c(tc, con, "bq_rep", (128, POS))
        bk_rep = _c(tc, con, "bk_rep", (128, POS))
        bv_rep = _c(tc, con, "bv_rep", (128, POS))
        bo_rep = _c(tc, con, "bo_rep", (128, POS))
        WvcaT = _c(tc, con, "WvcaT", (POS, POS))
        bvca = _c(tc, con, "bvca", (POS, 1))
        WocaT = _c(tc, con, "WocaT", (POS, POS))
        cab_rep = _c(tc, con, "cab_rep", (128, POS))
        g1_rep = _c(tc, con, "g1_rep", (128, POS))
        W1T = _c(tc, con, "W1T", (POS, FF))
        b1ff = _c(tc, con, "b1ff", (FF, 1))
        g2_rep = _c(tc, con, "g2_rep", (128, POS))
        W2T = _c(tc, con, "W2T", (FF, POS))
        b2_rep = _c(tc, con, "b2_rep", (128, POS))
        g3_rep = _c(tc, con, "g3_rep", (128, POS))
        b3_rep = _c(tc, con, "b3_rep", (128, POS))
        gon_rep = _c(tc, con, "gon_rep", (128, POS))
        bon_rep = _c(tc, con, "bon_rep", (128, POS))

        WeqT = _c(tc, con, "WeqT", (POS, POS))
        WekT = _c(tc, con, "WekT", (POS, POS))
        WevT = _c(tc, con, "WevT", (POS, POS))
        WeoT = _c(tc, con, "WeoT", (POS, POS))
        beq = _c(tc, con, "beq", (POS, 1))
        bek = _c(tc, con, "bek", (POS, 1))
        bev_rep = _c(tc, con, "bev_rep", (128, POS))
        beo = _c(tc, con, "beo", (POS, 1))
        We1T = _c(tc, con, "We1T", (POS, POS))
        be1ff = _c(tc, con, "be1ff", (POS, 1))
        We2T = _c(tc, con, "We2T", (POS, POS))
        be2ff = _c(tc, con, "be2ff", (POS, 1))

        ctx_glob = tc.tile_pool(name="glob", bufs=1)
        glob = ctx_glob.__enter__()
        topoT = [glob.tile([128, NCOL], F32, name=f"topoT{d}") for d in range(2)]

        def cols(tt, t):
            return tt * (T * SP) + t * SP

        def mm(out, lhsT, rhs, start, stop):
            PE.matmul(out, lhsT, rhs, start=start, stop=stop)

        def layernorm(pool, src, name):
            bst = pool.tile([128, 6], F32, name=name + "_bst", tag="ln_bst")
            bag = pool.tile([128, 2], F32, name=name + "_bag", tag="ln_bag")
            sd = pool.tile([128, 1], F32, name=name + "_sd", tag="ln_sd")
            inv = pool.tile([128, 1], F32, name=name + "_inv", tag="ln_inv")
            z = pool.tile([128, POS], F32, name=name + "_z", tag=name + "_z", bufs=1)
            V.bn_stats(bst[:, :], src)
            V.bn_aggr(bag[:, :], bst[:, :])
            A.activation(sd[:, :], bag[:, 1:2], AF.Sqrt, bias=eps_ap[:, 0:1])
            V.reciprocal(inv[:, :], sd[:, :])
            V.tensor_scalar(z[:, :], src, bag[:, 0:1], inv[:, :],
                            OP.subtract, OP.mult)
            return z

        # ---------------- prologue ----------------
        mem_tok = [glob.tile([128, POS], F32, name=f"memtok{tt}")
                   for tt in range(2)]
        ca_sb = [glob.tile([128, POS], F32, name=f"ca{tt}") for tt in range(2)]
        with tc.tile_pool(name="pro", bufs=2) as pro, \
                tc.tile_pool(name="prop", bufs=2, space="PSUM") as prop:
            for tt in range(2):
                gb = prop.tile([128, 2 * POS], F32, name="gb", tag="pp")
                mm(gb[:, :], fT[:, tt * 128:(tt + 1) * 128], se_wT[:, :],
                   True, True)
                tg = pro.tile([128, POS], F32, name="tg", tag="tg")
                tb = pro.tile([128, POS], F32, name="tb", tag="tb")
                V.tensor_tensor(tg[:, :], gb[:, :POS], se_bg[:, :], OP.add)
                V.tensor_tensor(tg[:, :], tg[:, :], xe[tt][:, :], OP.mult)
                V.tensor_tensor(tb[:, :], gb[:, POS:], se_bb[:, :], OP.add)
                V.tensor_tensor(tg[:, :], tg[:, :], tb[:, :], OP.add)
                sh = pro.tile([128, POS], F32, name="sh", tag="sh")
                A.activation(sh[:, :], tg[:, :], AF.Tanh)
                td = prop.tile([128, POS], F32, name="td", tag="pp")
                mm(td[:, :], cosC[:, :], sh[:, :], True, True)
                A.activation(mem_tok[tt][:, :], td[:, :], AF.Identity,
                             bias=zcol[:, 0:1])
                for d in range(2):
                    tp = prop.tile([128, 128], F32, name="tp", tag="pp")
                    PE.transpose(tp[:, :], mem_tok[tt][:, d * 128:(d + 1) * 128],
                                 ident[:, :])
                    c0 = cols(tt, 0)
                    A.activation(topoT[d][:, c0:c0 + SL], tp[:, :], AF.Identity,
                                 bias=zcol[:, 0:1])
            for d in range(2):
                dst = topoT[d][:, :].rearrange("p (b t a) -> p b t a",
                                               b=BL, t=T, a=SP)[:, :, :, SL:SP]
                nc.sync.dma_start(
                    dst, goal_d[d * 128:(d + 1) * 128, :, :].unsqueeze(3))
            vmem = []
            for d in range(2):
                vp = prop.tile([128, POS], F32, name="vp", tag="pp")
                for b in range(2):
                    for dj in range(2):
                        mm(vp[:, b * 128:(b + 1) * 128],
                           WvcaT[dj][:, d * 128:(d + 1) * 128],
                           topoT[dj][:, cols(b, 0):cols(b, 0) + SL],
                           dj == 0, dj == 1)
                vm = pro.tile([128, POS], F32, name="vm", tag=f"vm{d}")
                A.activation(vm[:, :], vp[:, :], AF.Identity,
                             bias=bvca[d][:, 0:1])
                vmem.append(vm)
            for tt in range(2):
                cp = prop.tile([128, POS], F32, name="cp", tag="pp")
                for d in range(2):
                    mm(cp[:, :], vmem[d][:, tt * 128:(tt + 1) * 128],
                       WocaT[d][:, :], d == 0, d == 1)
                V.tensor_tensor(ca_sb[tt][:, :], cp[:, :], cab_rep[:, :], OP.add)

        # ---------------- decoder ----------------
        with tc.tile_pool(name="decg", bufs=1) as decg, \
                tc.tile_pool(name="dec", bufs=2) as dec, \
                tc.tile_pool(name="decp", bufs=3, space="PSUM") as decp:
            kcache = [decg.tile([128, (T - 1) * POS], F32, name=f"kc{tt}")
                      for tt in range(2)]
            vcache = [decg.tile([128, (T - 1) * POS], F32, name=f"vc{tt}")
                      for tt in range(2)]
            oT = [decg.tile([128, POS], F32, name=f"oT{d}") for d in range(2)]
            z2T = [decg.tile([128, POS], F32, name=f"z2T{d}") for d in range(2)]
            r_prev = list(mem_tok)
            for i in range(1, T):
                z2s = []
                for tt in range(2):
                    xc = cols(tt, i - 1)
                    psq = decp.tile([128, POS], F32, name="psq", tag="qkv")
                    psk = decp.tile([128, POS], F32, name="psk", tag="qkv")
                    psv = decp.tile([128, POS], F32, name="psv", tag="qkv")
                    for d in range(2):
                        lx = topoT[d][:, xc:xc + SL]
                        mm(psq[:, :], lx, WqT[d][:, :], d == 0, d == 1)
                        mm(psk[:, :], lx, WkT[d][:, :], d == 0, d == 1)
                        mm(psv[:, :], lx, WvT[d][:, :], d == 0, d == 1)
                    q_sb = dec.tile([128, POS], F32, name="q_sb", tag="q_sb", bufs=1)
                    V.tensor_tensor(q_sb[:, :], psq[:, :], bq_rep[:, :], OP.add)
                    ks = kcache[tt][:, (i - 1) * POS:i * POS]
                    vs = vcache[tt][:, (i - 1) * POS:i * POS]
                    V.tensor_tensor(ks, psk[:, :], bk_rep[:, :], OP.add)
                    V.tensor_tensor(vs, psv[:, :], bv_rep[:, :], OP.add)

                    prod = dec.tile([128, T * POS], F32, name="prod", tag="pq", bufs=1)
                    qb = q_sb[:, :].unsqueeze(1).broadcast_to((128, i, POS))
                    V.tensor_tensor(
                        prod[:, :i * POS].rearrange("p (j f) -> p j f", j=i),
                        kcache[tt][:, :i * POS].rearrange("p (j f) -> p j f",
                                                          j=i),
                        qb, OP.mult)
                    sc = dec.tile([128, T * H], F32, name="sc", tag="sc")
                    V.tensor_reduce(
                        sc[:, :i * H],
                        prod[:, :i * POS].rearrange("p (g w) -> p g w", w=DH),
                        AX.X, OP.add)
                    pe_ = dec.tile([128, T * H], F32, name="pe_", tag="pe_")
                    A.activation(pe_[:, :i * H], sc[:, :i * H], AF.Exp)
                    den = dec.tile([128, H], F32, name="den", tag="den")
                    V.tensor_reduce(
                        den[:, :],
                        pe_[:, :i * H].rearrange("p (j h) -> p h j", h=H),
                        AX.X, OP.add)
                    rec = dec.tile([128, H], F32, name="rec", tag="rec")
                    V.reciprocal(rec[:, :], den[:, :])
                    pn = dec.tile([128, T * H], F32, name="pn", tag="pn")
                    rb = rec[:, :].unsqueeze(1).broadcast_to((128, i, H))
                    V.tensor_tensor(
                        pn[:, :i * H].rearrange("p (j h) -> p j h", h=H),
                        pe_[:, :i * H].rearrange("p (j h) -> p j h", h=H),
                        rb, OP.mult)
                    pv = dec.tile([128, T * POS], F32, name="pv", tag="pq", bufs=1)
                    pnb = pn[:, :i * H].rearrange("p (j h) -> p h j", h=H) \
                        .unsqueeze(2).broadcast_to((128, H, DH, i))
                    G.tensor_tensor(
                        pv[:, :i * POS].rearrange("p (h e j) -> p h e j",
                                                  h=H, e=DH),
                        vcache[tt][:, :i * POS].rearrange(
                            "p (j h e) -> p h e j", h=H, e=DH),
                        pnb, OP.mult)
                    o_sb = dec.tile([128, POS], F32, name="o_sb", tag="o_sb")
                    V.tensor_reduce(
                        o_sb[:, :],
                        pv[:, :i * POS].rearrange("p (f j) -> p f j", j=i),
                        AX.X, OP.add)
                    for d in range(2):
                        tp = decp.tile([128, 128], F32, name="tp2", tag="A")
                        PE.transpose(tp[:, :], o_sb[:, d * 128:(d + 1) * 128],
                                     ident[:, :])
                        A.activation(oT[d][:, tt * 128:(tt + 1) * 128],
                                     tp[:, :], AF.Identity, bias=zcol[:, 0:1])
                    sa = decp.tile([128, POS], F32, name="sa", tag="A")
                    for d in range(2):
                        mm(sa[:, :], oT[d][:, tt * 128:(tt + 1) * 128],
                           WoT[d][:, :], d == 0, False)
                    mm(sa[:, :], ident[:, :], bo_rep[:, :], False, False)
                    mm(sa[:, :], ident[:, :], r_prev[tt][:, :], False, True)
                    z1 = layernorm(dec, sa[:, :], "ln1")
                    t2 = dec.tile([128, POS], F32, name="t2", tag="t2", bufs=1)
                    V.tensor_tensor(t2[:, :], z1[:, :], g1_rep[:, :], OP.mult)
                    V.tensor_tensor(t2[:, :], t2[:, :], ca_sb[tt][:, :], OP.add)
                    z2 = layernorm(dec, t2[:, :], f"ln2_{tt}")
                    z2s.append(z2)
                    for d in range(2):
                        tp = decp.tile([128, 128], F32, name="tp3", tag="A")
                        PE.transpose(tp[:, :], z2[:, d * 128:(d + 1) * 128],
                                     ident[:, :])
                        A.activation(z2T[d][:, tt * 128:(tt + 1) * 128],
                                     tp[:, :], AF.Identity, bias=zcol[:, 0:1])
                ff1r = []
                for f in range(8):
                    fp = decp.tile([128, POS], F32, name="fp", tag="A")
                    for d in range(2):
                        mm(fp[:, :], W1T[d][:, f * 128:(f + 1) * 128],
                           z2T[d][:, :], d == 0, d == 1)
                    fr = dec.tile([128, POS], F32, name="fr", tag=f"fr{f}", bufs=1)
                    A.activation(fr[:, :], fp[:, :], AF.Relu,
                                 bias=b1ff[f][:, 0:1])
                    ff1r.append(fr)
                for tt in range(2):
                    f2 = decp.tile([128, POS], F32, name="f2", tag="A")
                    for f in range(8):
                        mm(f2[:, :], ff1r[f][:, tt * 128:(tt + 1) * 128],
                           W2T[f][:, :], f == 0, False)
                    mm(f2[:, :], ident[:, :], b2_rep[:, :], False, True)
                    t3 = dec.tile([128, POS], F32, name="t3", tag="t3", bufs=1)
                    V.tensor_tensor(t3[:, :], z2s[tt][:, :], g2_rep[:, :],
                                    OP.mult)
                    V.tensor_tensor(t3[:, :], t3[:, :], f2[:, :], OP.add)
                    z3 = layernorm(dec, t3[:, :], "ln3")
                    x3 = dec.tile([128, POS], F32, name="x3", tag="x3", bufs=1)
                    V.tensor_tensor(x3[:, :], z3[:, :], g3_rep[:, :], OP.mult)
                    V.tensor_tensor(x3[:, :], x3[:, :], b3_rep[:, :], OP.add)
                    zon = layernorm(dec, x3[:, :], "lnon")
                    rnew = dec.tile([128, POS], F32, name="rnew", tag=f"rn{tt}")
                    V.tensor_tensor(rnew[:, :], zon[:, :], gon_rep[:, :],
                                    OP.mult)
                    V.tensor_tensor(rnew[:, :], rnew[:, :], bon_rep[:, :],
                                    OP.add)
                    r_prev[tt] = rnew
                    for d in range(2):
                        tp = decp.tile([128, 128], F32, name="tp4", tag="A")
                        PE.transpose(tp[:, :], rnew[:, d * 128:(d + 1) * 128],
                                     ident[:, :])
                        c0 = cols(tt, i)
                        A.activation(topoT[d][:, c0:c0 + SL], tp[:, :],
                                     AF.Identity, bias=zcol[:, 0:1])

        # ---------------- encoder ----------------
        with tc.tile_pool(name="encg", bufs=1) as encg, \
                tc.tile_pool(name="enc", bufs=2) as enc, \
                tc.tile_pool(name="encp", bufs=4, space="PSUM") as encp:
            stats1 = encg.tile([NSEQ, 2 * SP], F32, name="stats1")
            stats2 = encg.tile([NSEQ, 2 * SP], F32, name="stats2")
            xenc = [[None, None] for _ in range(NSEQ)]

            def seq_ln_stats(src0, src1, stats, s):
                sp_ = encp.tile([1, 2 * SP], F32, name="sp_", tag="sml")
                sq0 = enc.tile([128, SP], F32, name="sq0", tag="sq0")
                sq1 = enc.tile([128, SP], F32, name="sq1", tag="sq1")
                A.activation(sq0[:, :], src0, AF.Square)
                A.activation(sq1[:, :], src1, AF.Square)
                mm(sp_[:, 0:SP], ones128[:, :], src0, True, False)
                mm(sp_[:, 0:SP], ones128[:, :], src1, False, True)
                mm(sp_[:, SP:], ones128[:, :], sq0[:, :], True, False)
                mm(sp_[:, SP:], ones128[:, :], sq1[:, :], False, True)
                stg = enc.tile([1, 2 * SP], F32, name="stg", tag="stg")
                A.activation(stg[:, :], sp_[:, :], AF.Identity,
                             bias=zcol[0:1, 0:1])
                nc.sync.dma_start(stats[s:s + 1, :], stg[:, :])

            def stats_to_minv(stats, mtile, itile):
                V.tensor_scalar_mul(mtile[:, :], stats[:, 0:SP], 1.0 / POS)
                msq = enc.tile([NSEQ, SP], F32, name="msq", tag="msq")
                V.tensor_tensor(msq[:, :], mtile[:, :], mtile[:, :], OP.mult)
                var = enc.tile([NSEQ, SP], F32, name="var", tag="var")
                V.scalar_tensor_tensor(var[:, :], stats[:, SP:], 1.0 / POS,
                                       msq[:, :], OP.mult, OP.subtract)
                sdv = enc.tile([NSEQ, SP], F32, name="sdv", tag="sdv")
                A.activation(sdv[:, :], var[:, :], AF.Sqrt,
                             bias=eps_ap[:NSEQ, 0:1])
                V.reciprocal(itile[:, :], sdv[:, :])

            for s in range(NSEQ):
                b, t = divmod(s, T)
                c0 = cols(b, t)
                seq_ln_stats(topoT[0][:, c0:c0 + SP], topoT[1][:, c0:c0 + SP],
                             stats1, s)
            m1 = encg.tile([NSEQ, SP], F32, name="m1")
            i1 = encg.tile([NSEQ, SP], F32, name="i1")
            stats_to_minv(stats1, m1, i1)

            for s in range(NSEQ):
                b, t = divmod(s, T)
                c0 = cols(b, t)
                mi = enc.tile([1, 2 * SP], F32, name="mi", tag="mi")
                nc.sync.dma_start(mi[:, 0:SP], m1[s:s + 1, :])
                nc.sync.dma_start(mi[:, SP:], i1[s:s + 1, :])
                rep = encp.tile([128, 2 * SP], F32, name="rep", tag="sml")
                mm(rep[:, 0:SP], onesr[:, :], mi[:, 0:SP], True, True)
                mm(rep[:, SP:], onesr[:, :], mi[:, SP:], True, True)
                hs = []
                for d in range(2):
                    h = enc.tile([128, SP], F32, name="h", tag=f"h{d}")
                    V.tensor_tensor(h[:, :], topoT[d][:, c0:c0 + SP],
                                    rep[:, 0:SP], OP.subtract)
                    V.tensor_tensor(h[:, :], h[:, :], rep[:, SP:], OP.mult)
                    hs.append(h)
                qT, kT = [], []
                for d in range(2):
                    qp = encp.tile([128, SP], F32, name="qp", tag="sml")
                    kp = encp.tile([128, SP], F32, name="kp", tag="sml")
                    for dj in range(2):
                        mm(qp[:, :], WeqT[dj][:, d * 128:(d + 1) * 128],
                           hs[dj][:, :], dj == 0, dj == 1)
                        mm(kp[:, :], WekT[dj][:, d * 128:(d + 1) * 128],
                           hs[dj][:, :], dj == 0, dj == 1)
                    qs = enc.tile([128, SP], F32, name="qs", tag=f"qs{d}")
                    ks = enc.tile([128, SP], F32, name="ks", tag=f"ks{d}")
                    A.activation(qs[:, :], qp[:, :], AF.Identity,
                                 bias=beq[d][:, 0:1])
                    A.activation(ks[:, :], kp[:, :], AF.Identity,
                                 bias=bek[d][:, 0:1])
                    qT.append(qs)
                    kT.append(ks)
                vAp = encp.tile([128, POS], F32, name="vAp", tag="sml")
                vGp = encp.tile([1, POS], F32, name="vGp", tag="sml")
                for dj in range(2):
                    mm(vAp[:, :], hs[dj][:, 0:SL], WevT[dj][:, :],
                       dj == 0, dj == 1)
                    mm(vGp[:, :], hs[dj][:, SL:SP], WevT[dj][:, :],
                       dj == 0, dj == 1)
                vA = enc.tile([128, POS], F32, name="vA", tag="vA")
                vG = enc.tile([1, POS], F32, name="vG", tag="vG")
                V.tensor_tensor(vA[:, :], vAp[:, :], bev_rep[:, :], OP.add)
                V.tensor_tensor(vG[:, :], vGp[:, :], bev_rep[0:1, :], OP.add)
                scp = encp.tile([128, 1024], F32, name="scp", tag="big", bufs=1)
                scg = encp.tile([1, 1024], F32, name="scg", tag="scg", bufs=1)
                for h in range(EH):
                    d, r0 = divmod(h * EDH, 128)
                    lk = kT[d][r0:r0 + EDH, :]
                    lq = qT[d][r0:r0 + EDH, :]
                    mm(scp[:, h * 256:h * 256 + SP], lk[:, 0:SL], lq,
                       True, True)
                    mm(scg[:, h * 256:h * 256 + SP], lk[:, SL:SP], lq,
                       True, True)
                probs = enc.tile([128, EH * SP], F32, name="probs", tag="probs")
                pg = enc.tile([1, EH * SP], F32, name="pg", tag="pg")
                A.activation(probs[:, :].rearrange("p (h q) -> p h q", q=SP),
                             scp[:, :].rearrange("p (h q) -> p h q", q=256)
                             [:, :, 0:SP], AF.Exp, bias=maskA[:, s:s + 1])
                A.activation(pg[:, :].rearrange("p (h q) -> p h q", q=SP),
                             scg[:, :].rearrange("p (h q) -> p h q", q=256)
                             [:, :, 0:SP], AF.Exp, bias=zcol[0:1, 0:1])
                dn = encp.tile([1, 2 * SP], F32, name="dn", tag="sml")
                dn2 = encp.tile([1, 2 * SP], F32, name="dn2", tag="sml")
                rraw = enc.tile([1, EH * SP], F32, name="rraw", tag="rraw")
                for p, dtile in enumerate((dn, dn2)):
                    mm(dtile[:, :], ones128[:, :],
                       probs[:, p * 2 * SP:(p + 1) * 2 * SP], True, False)
                    mm(dtile[:, :], ones128[0:1, 0:1],
                       pg[:, p * 2 * SP:(p + 1) * 2 * SP], False, True)
                    A.activation(rraw[:, p * 2 * SP:(p + 1) * 2 * SP],
                                 dtile[:, :], AF.Identity, bias=zcol[0:1, 0:1])
                rcb = enc.tile([1, EH * SP], F32, name="rcb", tag="rcb")
                V.reciprocal(rcb[:, :], rraw[:, :])
                rp = encp.tile([128, 1024], F32, name="rp", tag="big", bufs=1)
                for h in range(EH):
                    mm(rp[:, h * 256:h * 256 + SP], onesr[:, :],
                       rcb[:, h * SP:(h + 1) * SP], True, True)
                psc = enc.tile([128, EH * SP], F32, name="psc", tag="psc")
                V.tensor_tensor(psc[:, :].rearrange("p (h q) -> p h q", q=SP),
                                probs[:, :].rearrange("p (h q) -> p h q", q=SP),
                                rp[:, :].rearrange("p (h q) -> p h q", q=256)
                                [:, :, 0:SP], OP.mult)
                pgs = enc.tile([1, EH * SP], F32, name="pgs", tag="pgs")
                V.tensor_tensor(pgs[:, :], pg[:, :], rcb[:, :], OP.mult)
                oTs = []
                for p in range(2):
                    op_ = encp.tile([128, SP], F32, name="op_", tag="sml")
                    for hh in range(2):
                        h = 2 * p + hh
                        PE.matmul(op_[hh * EDH:(hh + 1) * EDH, :],
                                  vA[:, h * EDH:(h + 1) * EDH],
                                  psc[:, h * SP:(h + 1) * SP],
                                  start=True, stop=False,
                                  tile_position=(0, hh * EDH))
                        PE.matmul(op_[hh * EDH:(hh + 1) * EDH, :],
                                  vG[:, h * EDH:(h + 1) * EDH],
                                  pgs[:, h * SP:(h + 1) * SP],
                                  start=False, stop=True,
                                  tile_position=(0, hh * EDH))
                    os_ = enc.tile([128, SP], F32, name="os_", tag=f"os{p}")
                    A.activation(os_[:, :], op_[:, :], AF.Identity,
                                 bias=zcol[:, 0:1])
                    oTs.append(os_)
                for d in range(2):
                    yp = encp.tile([128, SP], F32, name="yp", tag="sml")
                    for kt in range(2):
                        mm(yp[:, :], WeoT[kt][:, d * 128:(d + 1) * 128],
                           oTs[kt][:, :], kt == 0, kt == 1)
                    xe_ = encg.tile([128, SP], F32, name=f"xenc{s}_{d}")
                    V.scalar_tensor_tensor(xe_[:, :], yp[:, :], beo[d][:, 0:1],
                                           topoT[d][:, c0:c0 + SP],
                                           OP.add, OP.add)
                    xenc[s][d] = xe_
                seq_ln_stats(xenc[s][0][:, :], xenc[s][1][:, :], stats2, s)

            m2 = encg.tile([NSEQ, SP], F32, name="m2")
            i2 = encg.tile([NSEQ, SP], F32, name="i2")
            stats_to_minv(stats2, m2, i2)

            for s in range(NSEQ):
                b, t = divmod(s, T)
                c0 = cols(b, t)
                mi2 = enc.tile([1, 2 * SP], F32, name="mi2", tag="mi2")
                nc.sync.dma_start(mi2[:, 0:SP], m2[s:s + 1, :])
                nc.sync.dma_start(mi2[:, SP:], i2[s:s + 1, :])
                rep = encp.tile([128, 2 * SP], F32, name="rep2", tag="sml")
                mm(rep[:, 0:SP], onesr[:, :], mi2[:, 0:SP], True, True)
                mm(rep[:, SP:], onesr[:, :], mi2[:, SP:], True, True)
                h2 = []
                for d in range(2):
                    h = enc.tile([128, SP], F32, name="h2", tag=f"h2{d}")
                    V.tensor_tensor(h[:, :], xenc[s][d][:, :], rep[:, 0:SP],
                                    OP.subtract)
                    V.tensor_tensor(h[:, :], h[:, :], rep[:, SP:], OP.mult)
                    h2.append(h)
                f1r = []
                for d in range(2):
                    f1p = encp.tile([128, SP], F32, name="f1p", tag="sml")
                    for dj in range(2):
                        mm(f1p[:, :], We1T[dj][:, d * 128:(d + 1) * 128],
                           h2[dj][:, :], dj == 0, dj == 1)
                    fr = enc.tile([128, SP], F32, name="fr2", tag=f"fr2{d}")
                    A.activation(fr[:, :], f1p[:, :], AF.Relu,
                                 bias=be1ff[d][:, 0:1])
                    f1r.append(fr)
                for d in range(2):
                    y2 = encp.tile([128, SP], F32, name="y2", tag="sml")
                    for dj in range(2):
                        mm(y2[:, :], We2T[dj][:, d * 128:(d + 1) * 128],
                           f1r[dj][:, :], dj == 0, dj == 1)
                    of = enc.tile([128, SP], F32, name="of", tag=f"of{d}")
                    V.scalar_tensor_tensor(of[:, :], y2[:, :], be2ff[d][:, 0:1],
                                           xenc[s][d][:, :], OP.add, OP.add)
                    nc.sync.dma_start(out_d[d * 128:(d + 1) * 128, c0:c0 + SP],
                                      of[:, :])
        ctx_glob.__exit__(None, None, None)
        ctx_con.__exit__(None, None, None)
    nc.compile()
    return nc


def _rep(v):
    return np.tile(np.asarray(v, np.float32)[None, :], (128, 1))


def _prep_shared(params):
    p = {k: np.asarray(v, np.float32) for k, v in params.items()}
    sc = 1.0 / math.sqrt(DH)
    wq, wk, wv = np.split(p["d_sa_wi"], 3, 0)
    bq, bk, bv = np.split(p["d_sa_bi"], 3, 0)
    n, k = np.meshgrid(np.arange(SL), np.arange(SL), indexing="ij")
    cosC = (np.cos(2 * np.pi * n * k / SL) / SL).astype(np.float32)
    esc = 1.0 / math.sqrt(EDH)
    ewq, ewk, ewv = np.split(p["e_sa_wi"], 3, 0)
    ebq, ebk, ebv = np.split(p["e_sa_bi"], 3, 0)
    ge1, be1 = p["e_n1_g"], p["e_n1_b"]
    ewq_f, ewk_f, ewv_f = ewq * ge1, ewk * ge1, ewv * ge1
    ebq_f = ebq + ewq @ be1
    ebk_f = ebk + ewk @ be1
    ebv_f = ebv + ewv @ be1
    ge2, be2 = p["e_n2_g"], p["e_n2_b"]
    we1_f = p["e_l1_w"] * ge2[None, :]
    be1ff = p["e_l1_b"] + p["e_l1_w"] @ be2
    wvca = p["d_ca_wi"][2 * POS:3 * POS]
    bvca = p["d_ca_bi"][2 * POS:3 * POS]
    d = {
        "se_wT": p["se_w"].T, "se_bg": _rep(p["se_b"][:POS]),
        "se_bb": _rep(p["se_b"][POS:]), "cosC": cosC,
        "ident": np.eye(128, dtype=np.float32),
        "ones128": np.ones((128, 1), np.float32),
        "onesr": np.ones((1, 128), np.float32),
        "eps_ap": np.full((128, 1), EPS, np.float32),
        "zcol": np.zeros((128, 1), np.float32),
        "WqT": wq.T * sc, "WkT": wk.T, "WvT": wv.T, "WoT": p["d_sa_wo"].T,
        "bq_rep": _rep(bq * sc), "bk_rep": _rep(bk), "bv_rep": _rep(bv),
        "bo_rep": _rep(p["d_sa_bo"]),
        "WvcaT": wvca.T, "bvca": bvca[:, None],
        "WocaT": p["d_ca_wo"].T,
        "cab_rep": _rep(p["d_ca_bo"] + p["d_n1_b"]),
        "g1_rep": _rep(p["d_n1_g"]),
        "W1T": (p["d_l1_w"] * p["d_n2_g"][None, :]).T,
        "b1ff": (p["d_l1_b"] + p["d_l1_w"] @ p["d_n2_b"])[:, None],
        "g2_rep": _rep(p["d_n2_g"]), "W2T": p["d_l2_w"].T,
        "b2_rep": _rep(p["d_l2_b"] + p["d_n2_b"]),
        "g3_rep": _rep(p["d_n3_g"]), "b3_rep": _rep(p["d_n3_b"]),
        "gon_rep": _rep(p["on_g"]), "bon_rep": _rep(p["on_b"]),
        "WeqT": ewq_f.T * esc, "WekT": ewk_f.T, "WevT": ewv_f.T,
        "WeoT": p["e_sa_wo"].T,
        "beq": (ebq_f * esc)[:, None], "bek": ebk_f[:, None],
        "bev_rep": _rep(ebv_f), "beo": p["e_sa_bo"][:, None],
        "We1T": we1_f.T, "be1ff": be1ff[:, None],
        "We2T": p["e_l2_w"].T, "be2ff": p["e_l2_b"][:, None],
    }
    return {k: np.ascontiguousarray(v, np.float32) for k, v in d.items()}


def kernel(agent_emb, agent_feature, goal_emb, agent_mask, params):
    agent_emb = np.asarray(agent_emb, np.float32)
    agent_feature = np.asarray(agent_feature, np.float32)
    goal_emb = np.asarray(goal_emb, np.float32)
    agent_mask = np.asarray(agent_mask)
    if "nc" not in _CACHE:
        _CACHE["nc"] = build_kernel()
    nc = _CACHE["nc"]
    shared = _prep_shared(params)
    in_maps = []
    for c in range(NCORES):
        b0 = BL * c
        vals = dict(shared)
        vals["xe_tok"] = agent_emb[b0:b0 + BL].reshape(S, POS)
        vals["fT"] = agent_feature[b0:b0 + BL].reshape(S, FEAT).T
        km = np.where(agent_mask[b0:b0 + BL], 0.0, NEG).astype(np.float32)
        vals["maskA"] = km.transpose(2, 0, 1).reshape(SL, NSEQ)
        m = {
            "cpack": _build_pack(vals),
            "goalT_bc": np.ascontiguousarray(
                np.repeat(goal_emb[b0:b0 + BL].T[:, :, None], T, axis=2)),
        }
        in_maps.append(m)
    res = run_bass_kernel_spmd(nc, in_maps, core_ids=list(range(NCORES)))
    outs = []
    for c in range(NCORES):
        o = np.asarray(res.results[c]["out_T"])
        outs.append(o.T.reshape(BL, T, SP, POS))
    return np.concatenate(outs, 0).astype(np.float32)


if __name__ == "__main__":
    build_kernel()
    print("build ok")
